# revision 24
# baseline (speedup 1.0000x reference)
"""Trainium2 Bass kernel for GatedCrossAttention (B=4, N=4096, C=1024, H=16, M=4).

Reference math (dead code removed: the v/gate projections are overwritten
by views of k in the original module, so v = g = k):
    q = query @ Wq.T + bq                    [B,N,C]   -> [B,N,H,hd]
    k = key   @ Wk.T + bk                    [B,N,M,C] -> [B,N,M,H,hd]
    attn = softmax_M(SCALE * einsum('bnhc,bnmhc->bnmh', q, k))
    out  = einsum('bnmh,bnmhc->bnhc', attn, k*k) . reshape(B,N,C)
    out  = out @ Wo.T + bo

Strategy: pure data parallel over the 16384 tokens (8 cores x 2048), no
collectives.  On-chip layout is "transposed": channels on partitions, tokens
on the free axis, so every matmul contraction (over channels) is a natural
PE op.  The projected channel space is HEAD-INTERLEAVED (new channel
c = d*16 + h, done host-side by permuting Wq/Wk output columns and Wo input
rows): every 128-partition tile then holds all 16 heads, so

  * the per-head logit reduction becomes: sum the 8 per-tile q*k products
    (DVE f16 chains, one accumulator per m-pair) into R, then 16 tiny
    indicator matmuls (stationary = R token chunk, moving = [128,16]
    head indicator) produce token-major logits for a full-width softmax;
  * the softmax-weight head->channel broadcast is ONE [64,128] indicator
    matmul per m (N=512) valid for all 8 channel tiles.

k^2 runs on the scalar engine (NOT GpSimd: GpSimd shares DVE's second
SBUF port pair and serializes against it); the weighted sum is f16 DVE
mul/add chains over m-PAIR tiles (half the instruction count).  The K
projection is m-outer with per-(r,m-pair) kp tiles so the q*k chains
accumulate DURING the projection stream.  Blocks of 512 tokens are
software-pipelined via deep tile pools, and the output projection has its
own PSUM pool so its (slow, yb-gated) accumulation groups never block the
next block's projection matmuls — the PE stays dense and warm (HAM at
2.4GHz).  Startup DMAs are ordered first-need-first with the Wq/Wk loads
split per output chunk, so the first matmul fires within a few us.  Host
pre-transposes/casts inputs and weights to fp16 (error vs f32 reference
~1e-3), accumulation stays f32.
"""

import dataclasses
import numpy as np
from contextlib import ExitStack

try:
    import concourse.bass as bass
except ImportError:  # path fallback for bare containers
    import sys

    sys.path.insert(0, "/opt/trn_rl_repo")
    import concourse.bass as bass

import concourse.tile as tile
from concourse import bacc, mybir
from concourse.bass_utils import run_bass_kernel_spmd
from concourse.masks import make_identity

# problem constants (hardcoded per the task contract)
B, N, C, H, HD, M = 4, 4096, 1024, 16, 64, 4
SCALE = float(HD) ** -0.5
NCORES = 8
T_TOTAL = B * N
T_CORE = T_TOTAL // NCORES  # 2048
TB = 512                    # tokens per block (one PSUM bank at f32)
NJ = C // 128               # 8 channel tiles
NT = TB // 128              # 4 token subtiles per block
MP = M // 2                 # m-pairs

DT = mybir.dt.float16
NPDT = np.float16
F32 = mybir.dt.float32


def _bcast(ap, reps, axis):
    """Insert a 0-stride dim of size `reps` at AP position `axis` (0=partition)."""
    new = list(ap.ap)
    new.insert(axis, [0, reps])
    return dataclasses.replace(ap, ap=new)


def build_nc(t_core=T_CORE, with_bias=False):
    nblk = t_core // TB
    nc = bacc.Bacc("TRN2", target_bir_lowering=False, debug=False)

    qT = nc.declare_dram_parameter("qT", [C, t_core], DT, isOutput=False)
    kT = nc.declare_dram_parameter("kT", [M, C, t_core], DT, isOutput=False)
    wqT = nc.declare_dram_parameter("wqT", [NJ, 128, NJ, 128], DT, isOutput=False)
    wkT = nc.declare_dram_parameter("wkT", [NJ, 128, NJ, 128], DT, isOutput=False)
    woT = nc.declare_dram_parameter("woT", [128, NJ, C], DT, isOutput=False)
    ind16 = nc.declare_dram_parameter("ind16", [128, H], DT, isOutput=False)
    indw = nc.declare_dram_parameter("indw", [64, M, 128], DT, isOutput=False)
    if with_bias:
        bq = nc.declare_dram_parameter("bq", [1, C], DT, isOutput=False)
        bk = nc.declare_dram_parameter("bk", [1, C], DT, isOutput=False)
        bo = nc.declare_dram_parameter("bo", [1, C], DT, isOutput=False)
    out = nc.declare_dram_parameter("out", [t_core, C], F32, isOutput=True)

    # DRAM views: channel dim split into (chunk, partition)
    qT_v = qT.ap().rearrange("(c p) t -> p c t", p=128)
    kT_v = kT.ap().rearrange("m (c p) t -> p m c t", p=128)

    with tile.TileContext(nc) as tc, ExitStack() as ctx:
        consts = ctx.enter_context(tc.tile_pool(name="consts", bufs=1))
        p_inq = ctx.enter_context(tc.tile_pool(name="inq", bufs=2))
        p_ink = ctx.enter_context(tc.tile_pool(name="ink", bufs=2))
        p_qp = ctx.enter_context(tc.tile_pool(name="qp", bufs=16))
        p_kp = ctx.enter_context(tc.tile_pool(name="kp", bufs=20))
        p_R = ctx.enter_context(tc.tile_pool(name="R", bufs=2))
        p_tmp = ctx.enter_context(tc.tile_pool(name="tmp", bufs=2))
        p_ksq = ctx.enter_context(tc.tile_pool(name="ksq", bufs=17))
        p_sm = ctx.enter_context(tc.tile_pool(name="sm", bufs=1))
        p_wT = ctx.enter_context(tc.tile_pool(name="wT", bufs=2))
        p_wbc = ctx.enter_context(tc.tile_pool(name="wbc", bufs=1))
        p_ct = ctx.enter_context(tc.tile_pool(name="ct", bufs=2))
        p_yb = ctx.enter_context(tc.tile_pool(name="yb", bufs=8))
        p_out = ctx.enter_context(tc.tile_pool(name="outs", bufs=2))
        pp = ctx.enter_context(tc.tile_pool(name="pp", bufs=3, space="PSUM"))
        po = ctx.enter_context(tc.tile_pool(name="po", bufs=2, space="PSUM"))
        pl = ctx.enter_context(tc.tile_pool(name="pl", bufs=1, space="PSUM"))
        pw = ctx.enter_context(tc.tile_pool(name="pw", bufs=1, space="PSUM"))
        pb = ctx.enter_context(tc.tile_pool(name="pb", bufs=1, space="PSUM"))

        # ---- constants / inputs, first-need-first DMA order ----
        ind16_sb = consts.tile([128, H], DT)
        indw_sb = consts.tile([64, M, 128], DT)
        nc.sync.dma_start(out=ind16_sb, in_=ind16.ap())
        nc.sync.dma_start(out=indw_sb, in_=indw.ap())

        # block-0 query input before the weights: it gates the first matmul
        q_in0 = p_inq.tile([128, NJ, TB // 2], DT, tag="q", name="q",
                           padded_shape=[128, NJ, TB])
        for i in range(4):
            nc.sync.dma_start(out=q_in0[:, 2 * i:2 * i + 2, :],
                              in_=qT_v[:, 2 * i:2 * i + 2, 0:TB // 2])
        wq_sb = consts.tile([128, NJ, C], DT)
        for r in range(NJ):
            nc.sync.dma_start(out=wq_sb[:, :, r * 128:(r + 1) * 128],
                              in_=wqT.ap()[r])
        k_in0 = [p_ink.tile([128, NJ, TB // 2], DT, tag="kin", name="kin",
                            padded_shape=[128, NJ, TB])
                 for _ in range(M)]
        wk_sb = consts.tile([128, NJ, C], DT)
        wo_sb = consts.tile([128, NJ, C], DT)
        # stagger the rest so q_in0+wq get the DMA bandwidth first
        with tc.tile_wait_until(0.004):
            nc.sync.dma_start(out=k_in0[0], in_=kT_v[:, 0, :, 0:TB // 2])
            for r in range(NJ):
                nc.sync.dma_start(out=wk_sb[:, :, r * 128:(r + 1) * 128],
                                  in_=wkT.ap()[r])
        with tc.tile_wait_until(0.010):
            for m in range(1, M):
                nc.sync.dma_start(out=k_in0[m], in_=kT_v[:, m, :, 0:TB // 2])
        with tc.tile_wait_until(0.018):
            nc.sync.dma_start(out=wo_sb, in_=woT.ap())
        ident = consts.tile([128, 128], DT)
        make_identity(nc, ident)
        if with_bias:
            ones_sb = consts.tile([1, TB], DT)
            nc.vector.memset(ones_sb, 1.0)
            bq_sb = consts.tile([1, C], DT)
            bk_sb = consts.tile([1, C], DT)
            bo_sb = consts.tile([1, C], DT)
            nc.sync.dma_start(out=bq_sb, in_=bq.ap())
            nc.sync.dma_start(out=bk_sb, in_=bk.ap())
            nc.sync.dma_start(out=bo_sb, in_=bo.ap())

        # full 512-token blocks, with the LAST one split in half so the
        # second half's projections overlap the first half's attention
        # middle (shrinks the un-overlapped pipeline tail)
        blocks = [(0, TB // 2), (TB // 2, TB // 2)]
        blocks += [(i * TB, TB) for i in range(1, nblk - 1)]
        blocks += [(t_core - TB, TB // 2),
                   (t_core - TB // 2, TB // 4), (t_core - TB // 4, TB // 4)]

        for bi, (t0, tb) in enumerate(blocks):
            nt = tb // 128
            tsl = slice(t0, t0 + tb)

            # ---- load inputs (block 0 preloaded above) ----
            if bi == 0:
                q_in, k_in = q_in0, k_in0
            else:
                q_in = p_inq.tile([128, NJ, tb], DT, tag="q", name="q",
                                  padded_shape=[128, NJ, TB])
                nc.sync.dma_start(out=q_in, in_=qT_v[:, :, tsl])
                k_in = [p_ink.tile([128, NJ, tb], DT, tag="kin", name="kin",
                                   padded_shape=[128, NJ, TB])
                        for _ in range(M)]
                for m in range(M):
                    nc.sync.dma_start(out=k_in[m], in_=kT_v[:, m, :, tsl])

            # ---- Q projection (PE; psum evacuated by DVE) ----
            qp = [p_qp.tile([128, tb], DT, tag="qp", name="qp",
                            padded_shape=[128, TB]) for _ in range(NJ)]
            for r in range(NJ):
                ps = pp.tile([128, tb], F32, tag="pp", name="pp",
                             padded_shape=[128, TB])
                for c in range(NJ):
                    nc.tensor.matmul(
                        ps,
                        wq_sb[:, c, r * 128:(r + 1) * 128],
                        q_in[:, c, :],
                        start=(c == 0),
                        stop=(c == NJ - 1 and not with_bias),
                    )
                if with_bias:
                    nc.tensor.matmul(
                        ps, bq_sb[:, r * 128:(r + 1) * 128], ones_sb[:, :tb],
                        start=False, stop=True,
                    )
                nc.vector.tensor_copy(qp[r], ps)

            # ---- K projection (PE), m-outer; per-(r,m-pair) kp tiles so
            #      the q*k product chains accumulate during the stream ----
            kp = [[None] * MP for _ in range(NJ)]
            Rs = []
            for m in range(M):
                mp, ml = m // 2, m % 2
                for r in range(NJ):
                    ps = pp.tile([128, tb], F32, tag="pp", name="pp",
                                 padded_shape=[128, TB])
                    for c in range(NJ):
                        nc.tensor.matmul(
                            ps,
                            wk_sb[:, c, r * 128:(r + 1) * 128],
                            k_in[m][:, c, :],
                            start=(c == 0),
                            stop=(c == NJ - 1 and not with_bias),
                        )
                    if with_bias:
                        nc.tensor.matmul(
                            ps, bk_sb[:, r * 128:(r + 1) * 128],
                            ones_sb[:, :tb],
                            start=False, stop=True,
                        )
                    if ml == 0:
                        kp[r][mp] = p_kp.tile([128, 2, tb], DT, tag="kp",
                                              name="kp",
                                              padded_shape=[128, 2, TB])
                    nc.scalar.copy(out=kp[r][mp][:, ml, :], in_=ps)

                # R_mp = sum_r qp_r * kp_r_mp  (DVE f16, fires as copies land)
                if ml == 1:
                    Rm = p_R.tile([128, 2, tb], DT, tag="R", name="R",
                                  padded_shape=[128, 2, TB])
                    nc.vector.tensor_mul(Rm, _bcast(qp[0], 2, 1), kp[0][mp])
                    for r in range(1, NJ):
                        t = p_tmp.tile([128, 2, tb], DT, tag="tmp", name="tmp",
                                       padded_shape=[128, 2, TB])
                        nc.vector.tensor_mul(t, _bcast(qp[r], 2, 1), kp[r][mp])
                        nc.vector.tensor_add(Rm, Rm, t)
                    Rs.append(Rm)

            # ---- k^2 on ACT, hoisted so it runs inside the K window and
            #      kp slots free at block end (unblocks next K projection).
            #      For the LAST block it is emitted after wbc instead, so
            #      the squares don't delay exp/softmax in the pipeline tail.
            def emit_ksqs():
                ksqs = [[None] * MP for _ in range(NJ)]
                for r in range(NJ):
                    for mp in range(MP):
                        ksqs[r][mp] = p_ksq.tile([128, 2, tb], DT, tag="ksq",
                                                 name="ksq",
                                                 padded_shape=[128, 2, TB])
                        nc.scalar.activation(
                            ksqs[r][mp], kp[r][mp],
                            func=mybir.ActivationFunctionType.Square,
                        )
                return ksqs

            last = bi == len(blocks) - 1
            if not last:
                ksqs = emit_ksqs()

            # ---- attention logits, token-major: pslt[t, tt, m, h] ----
            pslt = pl.tile([128, nt, M, H], F32, tag="pl", name="pl",
                           padded_shape=[128, NT, M, H])
            for m in range(M):
                for tt in range(nt):
                    nc.tensor.matmul(
                        pslt[:, tt, m, :],
                        Rs[m // 2][:, m % 2, tt * 128:(tt + 1) * 128],
                        ind16_sb,
                        start=True,
                        stop=True,
                    )

            # ---- softmax over M (token-major, full 128 partitions) ----
            e = p_sm.tile([128, nt, M, H], F32, tag="e", name="e",
                          padded_shape=[128, NT, M, H])
            nc.scalar.activation(e, pslt, func=mybir.ActivationFunctionType.Exp)
            s01 = p_sm.tile([128, nt, H], F32, tag="s01", name="s01",
                            padded_shape=[128, NT, H])
            s = p_sm.tile([128, nt, H], F32, tag="s", name="s",
                          padded_shape=[128, NT, H])
            nc.vector.tensor_add(s01, e[:, :, 0, :], e[:, :, 1, :])
            nc.vector.tensor_add(s, e[:, :, 2, :], e[:, :, 3, :])
            nc.vector.tensor_add(s, s01, s)
            rcp = p_sm.tile([128, nt, H], F32, tag="rcp", name="rcp",
                            padded_shape=[128, NT, H])
            nc.vector.reciprocal(rcp, s)
            w_t = p_sm.tile([128, nt, M, H], DT, tag="w", name="w",
                            padded_shape=[128, NT, M, H])
            nc.vector.tensor_mul(w_t, e, _bcast(rcp, M, 2))

            # transpose w to head-major: wT[(m,h), (tt,t)]
            wT = p_wT.tile([64, nt, 128], DT, tag="wT", name="wT",
                           padded_shape=[64, NT, 128])
            for tt in range(nt):
                pst = pw.tile([64, 128], DT, tag="pw", name="pw")
                nc.tensor.transpose(pst, w_t[:, tt, :, :], ident)
                nc.scalar.copy(out=wT[:, tt, :], in_=pst)

            # ---- head->channel broadcast of softmax weights (PE, one MM per m) ----
            wbc = p_wbc.tile([128, M, tb], DT, tag="wbc", name="wbc",
                             padded_shape=[128, M, TB])
            for m in range(M):
                psb = pb.tile([128, tb], F32, tag="pb", name="pb",
                              padded_shape=[128, TB])
                nc.tensor.matmul(
                    psb, indw_sb[:, m, :], wT, start=True, stop=True,
                )
                nc.scalar.copy(out=wbc[:, m, :], in_=psb)

            if last:
                ksqs = emit_ksqs()

            # ---- weighted sum of k^2 (ACT square + DVE m-pair mul/adds) ----
            yb = [p_yb.tile([128, tb], DT, tag="yb", name="yb",
                            padded_shape=[128, TB]) for _ in range(NJ)]
            for r in range(NJ):
                u = [None, None]
                for mp in range(MP):
                    u[mp] = p_ct.tile([128, 2, tb], DT, tag="ct", name="ct",
                                      padded_shape=[128, 2, TB])
                    nc.vector.tensor_mul(
                        u[mp], wbc[:, 2 * mp:2 * mp + 2, :], ksqs[r][mp]
                    )
                nc.vector.tensor_add(u[0], u[0], u[1])
                nc.vector.tensor_add(yb[r], u[0][:, 0, :], u[0][:, 1, :])

            # ---- output projection (PE, own PSUM pool so the yb-gated
            #      accumulation groups never starve the projection stream) ----
            for tt in range(nt):
                for oc in range(2):
                    ps = po.tile([128, 512], F32, tag="po", name="po")
                    for r in range(NJ):
                        nc.tensor.matmul(
                            ps,
                            yb[r][:, tt * 128:(tt + 1) * 128],
                            wo_sb[:, r, oc * 512:(oc + 1) * 512],
                            start=(r == 0),
                            stop=(r == NJ - 1 and not with_bias),
                        )
                    if with_bias:
                        nc.tensor.matmul(
                            ps,
                            ones_sb[:, :128],
                            bo_sb[:, oc * 512:(oc + 1) * 512],
                            start=False,
                            stop=True,
                        )
                    o_sb = p_out.tile([128, 512], F32, tag="outs", name="osb")
                    nc.scalar.copy(out=o_sb, in_=ps)
                    nc.sync.dma_start(
                        out=out.ap()[t0 + tt * 128:t0 + (tt + 1) * 128,
                                     oc * 512:(oc + 1) * 512],
                        in_=o_sb,
                    )
    nc.compile()
    return nc


def _host_prep(query, key, Wq, Wk, Wo, bq, bk, bo):
    qT = np.ascontiguousarray(query.reshape(T_TOTAL, C).T).astype(NPDT)
    kT = np.ascontiguousarray(key.reshape(T_TOTAL, M, C).transpose(1, 2, 0)).astype(NPDT)

    # head-interleaved projection space: new channel c = d*16 + h
    cc = np.arange(C)
    old0 = (cc % H) * HD + cc // H
    def _rpcj(w):  # [in,out] -> [r, p, c, j] chunk-contiguous layout
        return np.ascontiguousarray(
            w.reshape(NJ, 128, NJ, 128).transpose(2, 1, 0, 3))

    wqT = _rpcj(np.asarray(Wq.T[:, old0], np.float32).astype(NPDT))
    wkT = _rpcj(np.asarray(Wk.T[:, old0], np.float32).astype(NPDT))
    woT = np.ascontiguousarray(
        np.asarray(Wo.T[old0, :], np.float32).astype(NPDT)
        .reshape(NJ, 128, C).transpose(1, 0, 2))

    p = np.arange(128)
    ind16 = (p[:, None] % H == np.arange(H)[None, :]).astype(NPDT) * NPDT(SCALE)
    # indw[q, m, p] = 1 iff q == m*H + (p % H)
    q_ = np.arange(64)[:, None, None]
    m_ = np.arange(M)[None, :, None]
    indw = (q_ == m_ * H + p[None, None, :] % H).astype(NPDT)

    with_bias = bool(np.any(bq) or np.any(bk) or np.any(bo))
    common = {"wqT": wqT, "wkT": wkT, "woT": woT, "ind16": ind16, "indw": indw}
    if with_bias:
        common |= {
            "bq": bq.reshape(1, C)[:, old0].astype(NPDT),
            "bk": bk.reshape(1, C)[:, old0].astype(NPDT),
            "bo": bo.reshape(1, C).astype(NPDT),
        }
    in_maps = []
    for i in range(NCORES):
        sl = slice(i * T_CORE, (i + 1) * T_CORE)
        in_maps.append(
            {
                "qT": np.ascontiguousarray(qT[:, sl]),
                "kT": np.ascontiguousarray(kT[:, :, sl]),
                **common,
            }
        )
    return in_maps, with_bias


_NC_CACHE = {}
_LAST_RESULT = None


def kernel(query, key, gate, Wq, bq, Wk, bk, Wv, bv, Wg, bg, Wo, bo):
    in_maps, with_bias = _host_prep(query, key, Wq, Wk, Wo, bq, bk, bo)
    key_ = (T_CORE, with_bias)
    if key_ not in _NC_CACHE:
        _NC_CACHE[key_] = build_nc(T_CORE, with_bias)
    nc = _NC_CACHE[key_]
    res = run_bass_kernel_spmd(nc, in_maps, list(range(NCORES)))
    global _LAST_RESULT
    _LAST_RESULT = res
    out = np.concatenate([res.results[i]["out"] for i in range(NCORES)], axis=0)
    return out.reshape(B, N, C)
